# revision 1
# baseline (speedup 1.0000x reference)
"""Data-parallel Trainium kernel for nn_InterpreterWithRegistersAndKbit.

Shards the batch dim (B=256) across 8 NeuronCores (32 elems/core); all
params replicated. The sequential 32-line scan runs independently per
batch shard on each core.
"""
import numpy as np
import jax
import jax.numpy as jnp
from functools import partial

B, LINES, CAT = 256, 32, 5
NREG, D = 64, 1024
L, K = 64, 16
EPS = 1e-5
NCORES = 8
BS = B // NCORES  # 32 per core


def _ln(x, g, b):
    m = x.mean(-1, keepdims=True)
    v = x.var(-1, keepdims=True)
    return (x - m) * jax.lax.rsqrt(v + EPS) * g + b


def _mod_tables():
    i = np.arange(K)[:, None, None]
    j = np.arange(K)[None, :, None]
    m = np.arange(K)[None, None, :]
    Tadd = ((i + j) % K == m).astype(np.float32)
    Tsub = ((i - j) % K == m).astype(np.float32)
    return jnp.asarray(Tadd), jnp.asarray(Tsub)


def _shard_body(opcode_probs, registers, k_write, q_read, gate,
                ln1_g, ln1_b, W1, b1, ln2_g, ln2_b, W2, b2,
                Wr, br, lnf_g, lnf_b):
    b = opcode_probs.shape[0]
    Tadd, Tsub = _mod_tables()
    scale = 1.0 / jnp.sqrt(jnp.float32(D))
    q = q_read.reshape(b, LINES, 2, D)

    def step(regs, xs):
        prob, ql, kw, g = xs
        attn = jax.nn.softmax(jnp.einsum('bqd,bnd->bqn', ql, regs) * scale, axis=-1)
        op_s = jnp.einsum('bqn,bnd->bqd', attn, regs)
        x = jax.nn.softmax((_ln(op_s[:, 0], ln1_g, ln1_b) @ W1 + b1).reshape(b, L, K), axis=-1)
        y = jax.nn.softmax((_ln(op_s[:, 1], ln2_g, ln2_b) @ W2 + b2).reshape(b, L, K), axis=-1)
        add = jnp.einsum('bli,blj,ijm->blm', x, y, Tadd)
        sub = jnp.einsum('bli,blj,ijm->blm', x, y, Tsub)
        outs = jnp.stack([jnp.zeros_like(x), x, y, add, sub], axis=1)
        mix = jnp.einsum('bn,bnlk->blk', prob, outs)
        value = _ln(mix.reshape(b, L * K) @ Wr + br, lnf_g, lnf_b)[:, None, :]
        wa = jax.nn.softmax(jnp.einsum('bod,bnd->bon', kw, regs) * scale, axis=-1)
        geff = g[:, :, None] * (1.0 - prob[:, 0:1, None])
        w = (wa * geff).transpose(0, 2, 1)
        regs = regs * (1.0 - w) + w * value
        return regs, None

    xs = (jnp.moveaxis(opcode_probs, 1, 0),
          jnp.moveaxis(q, 1, 0),
          jnp.moveaxis(k_write, 1, 0),
          jnp.moveaxis(gate, 1, 0))
    regs, _ = jax.lax.scan(step, registers, xs)
    return regs


_pmapped = None


def _get_pmapped():
    global _pmapped
    if _pmapped is None:
        devs = jax.devices()[:NCORES]
        _pmapped = jax.pmap(
            _shard_body,
            in_axes=(0, 0, 0, 0, 0) + (None,) * 12,
            devices=devs,
        )
    return _pmapped


def kernel(opcode_probs, registers, k_write, q_read, gate,
           ln1_g, ln1_b, W1, b1, ln2_g, ln2_b, W2, b2,
           Wr, br, lnf_g, lnf_b):
    f = _get_pmapped()

    def sh(a):  # shard batch dim across cores
        a = np.asarray(a)
        return a.reshape(NCORES, BS, *a.shape[1:])

    out = f(sh(opcode_probs), sh(registers), sh(k_write), sh(q_read), sh(gate),
            np.asarray(ln1_g), np.asarray(ln1_b), np.asarray(W1), np.asarray(b1),
            np.asarray(ln2_g), np.asarray(ln2_b), np.asarray(W2), np.asarray(b2),
            np.asarray(Wr), np.asarray(br), np.asarray(lnf_g), np.asarray(lnf_b))
    out = np.asarray(out).reshape(B, NREG, D).astype(np.float32)
    return out
